# revision 4
# baseline (speedup 1.0000x reference)
"""NaiveFourierKANLayer on 8 Trainium2 NeuronCores (Bass/Tile).

y[b,j] = sum_{i,g} cos(g*x[b,i]) * W[0,j,i,g] + sin(g*x[b,i]) * W[1,j,i,g]

Strategy (data-parallel over batch, 1024 rows/core):
- Host: range-reduce x to [-pi,pi] (g integer => g*x mod 2pi preserved),
  transpose to x^T [i,b]; pack W bf16 as per-contraction-tile slabs
  [phase, n, ki, all-j].
- Device per core: theta_g chain via one fused custom DVE op per harmonic
  (tensor-add + period-wrap), sin+cos args evaluated by a single ScalarE Sin
  pass per harmonic (bf16 out); TensorE runs 2048 accumulating matmuls
  (K=128, M=128 j, N=512 b) n-outer/j-inner into 8 PSUM banks; y^T out f32.
"""
import numpy as np
import ml_dtypes

import concourse.mybir as mybir
import concourse.tile as tile
from concourse import bacc
from concourse.bass_utils import run_bass_kernel_spmd

import fused_op

N_CORES = 8
B_TOTAL = 8192
B_LOCAL = B_TOTAL // N_CORES   # 1024
I_DIM = 1024
J_DIM = 1024
G = 8
P = 128
NB_HALF = 2                    # batch halves per core (512 cols each)
BH = B_LOCAL // NB_HALF        # 512
N_PHASE = 2                    # contraction phases (i-tiles 0-3, 4-7)
II_PER_PHASE = I_DIM // P // N_PHASE   # 4
NT = II_PER_PHASE * G * 2      # 64 contraction tiles per phase
NJ = J_DIM // P                # 8

PI = float(np.pi)
TWO_PI = float(2 * np.pi)
AF = mybir.ActivationFunctionType
BF16 = mybir.dt.bfloat16
F32 = mybir.dt.float32

_NC_CACHE = {}


def _emit_mms(nc, ps_tiles, wp, w_d, const_w, ft, bh, ph, n, variant):
    if variant == "mm_nodma":
        wt = const_w
    else:
        wt = wp.tile([P, J_DIM], BF16, tag="w", bufs=16,
                     name=f"w_{bh}_{ph}_{n}")
        nc.sync.dma_start(out=wt, in_=w_d[ph, n])
    for jt in range(NJ):
        nc.tensor.matmul(
            ps_tiles[jt], wt[:, jt * P:(jt + 1) * P], ft,
            start=(ph == 0 and n == 0),
            stop=(ph == N_PHASE - 1 and n == NT - 1),
        )


def _body(nc, tc, xp, wp, fp, tp, op, pp, xT_d, w_d, yT_d, variant="full"):
    xt = xp.tile([P, I_DIM // P, B_LOCAL], F32, name="xt")
    nc.sync.dma_start(out=xt, in_=xT_d.rearrange("(it p) b -> p it b", p=P))
    const_f = None
    const_w = None
    if variant in ("mm_only", "mm_nodma"):
        const_f = xp.tile([P, BH], BF16, name="const_f")
        nc.vector.memset(const_f, 0.5)
    if variant == "mm_nodma":
        const_w = xp.tile([P, J_DIM], BF16, name="const_w")
        nc.vector.memset(const_w, 0.5)

    for bh in range(NB_HALF):
        bs = slice(bh * BH, (bh + 1) * BH)
        ps_tiles = []
        for jt in range(NJ):
            ps = pp.tile([P, BH], F32, tag=f"ps{jt}", name=f"ps{jt}_{bh}")
            ps_tiles.append(ps)

        for ph in range(N_PHASE):
            # n-outer schedule: feature tile n is consumed by 8 back-to-back
            # matmuls (one per j-tile) right after production, so feat slots
            # recycle fast and ACT/DVE stay ahead of PE across boundaries.
            for ii in range(II_PER_PHASE):
                it = ph * II_PER_PHASE + ii
                xs = xt[:, it, bs]
                if variant in ("mm_only", "mm_nodma"):
                    for n in range(ii * 2 * G, (ii + 1) * 2 * G):
                        _emit_mms(nc, ps_tiles, wp, w_d, const_w, const_f,
                                  bh, ph, n, variant)
                    continue
                for g in range(1, G + 1):
                    nbase = ii * (G * 2) + (g - 1) * 2
                    if g == 1:
                        tcos = tp.tile([P, BH], F32, tag="tc", bufs=3,
                                       name=f"tc_{bh}_{it}")
                        nc.vector.add_range_wrap(tcos, xs, PI / 2, PI, TWO_PI)
                        fc = fp.tile([P, BH], BF16, tag=f"fc{ii}",
                                     name=f"fc_{bh}_{it}")
                        nc.scalar.activation(out=fc, in_=tcos, func=AF.Sin)
                        fs = fp.tile([P, BH], BF16, tag=f"fs{ii}",
                                     name=f"fs_{bh}_{it}")
                        nc.scalar.activation(out=fs, in_=xs, func=AF.Sin)
                        th_prev = xs
                        f_cos, f_sin = fc, fs
                    else:
                        tharg = tp.tile([P, 2, BH], F32, tag="th", bufs=6,
                                        name=f"th_{bh}_{it}_{g}")
                        fused_op.add_t_range_wrap(
                            nc, tharg[:, 1, :], th_prev, xs, PI, TWO_PI)
                        nc.vector.add_range_wrap(
                            tharg[:, 0, :], tharg[:, 1, :], PI / 2, PI, TWO_PI)
                        f = fp.tile([P, 2, BH], BF16, tag=f"f{ii}_{g}",
                                    name=f"f_{bh}_{it}_{g}")
                        nc.scalar.activation(out=f, in_=tharg, func=AF.Sin)
                        th_prev = tharg[:, 1, :]
                        f_cos, f_sin = f[:, 0, :], f[:, 1, :]
                    if variant == "feats_only":
                        continue
                    _emit_mms(nc, ps_tiles, wp, w_d, const_w, f_cos,
                              bh, ph, nbase, variant)
                    _emit_mms(nc, ps_tiles, wp, w_d, const_w, f_sin,
                              bh, ph, nbase + 1, variant)

        if variant != "feats_only":
            for jt in range(NJ):
                ot = op.tile([P, BH], F32, tag="out", name=f"ot_{bh}_{jt}")
                nc.scalar.copy(out=ot, in_=ps_tiles[jt])
                nc.sync.dma_start(out=yT_d[jt * P:(jt + 1) * P, bs], in_=ot)


def _build_nc(loop_reps=None, variant="full"):
    nc = bacc.Bacc("TRN2", debug=False, num_devices=N_CORES)
    xT_d = nc.dram_tensor("xT", [I_DIM, B_LOCAL], F32, kind="ExternalInput").ap()
    w_d = nc.dram_tensor("w", [N_PHASE, NT, P, J_DIM], BF16, kind="ExternalInput").ap()
    yT_d = nc.dram_tensor("yT", [J_DIM, B_LOCAL], F32, kind="ExternalOutput").ap()

    with tile.TileContext(nc) as tc:
        with tc.tile_pool(name="xp", bufs=1) as xp, \
             tc.tile_pool(name="wp", bufs=3) as wp, \
             tc.tile_pool(name="fp", bufs=1) as fp, \
             tc.tile_pool(name="tp", bufs=1) as tp, \
             tc.tile_pool(name="op", bufs=4) as op, \
             tc.tile_pool(name="pp", bufs=1, space="PSUM") as pp:
            pools = (xp, wp, fp, tp, op, pp)
            if loop_reps is None:
                _body(nc, tc, *pools, xT_d, w_d, yT_d, variant=variant)
            else:
                with tc.For_i(0, loop_reps, 1):
                    _body(nc, tc, *pools, xT_d, w_d, yT_d, variant=variant)

    nc.compile()
    return nc


def get_nc(loop_reps=None, variant="full"):
    key = (loop_reps, variant)
    if key not in _NC_CACHE:
        _NC_CACHE[key] = _build_nc(loop_reps, variant)
    return _NC_CACHE[key]


def prepare_inputs(x, fouriercoeffs):
    """Host-side prep: range-reduce + transpose x, pack W to bf16 slabs."""
    x = np.asarray(x, dtype=np.float32)
    w = np.asarray(fouriercoeffs, dtype=np.float32)
    x64 = x.astype(np.float64)
    x_red = (x64 - TWO_PI * np.round(x64 / TWO_PI)).astype(np.float32)
    # pack: coeffs [t, j, i, g] -> [ph, n=(ii,g,t), ki, j]
    a = w.reshape(2, J_DIM, N_PHASE, II_PER_PHASE, P, G)  # [t, j, ph, ii, ki, g]
    a = a.transpose(2, 3, 5, 0, 4, 1)                     # [ph, ii, g, t, ki, j]
    w_pack = np.ascontiguousarray(a.reshape(N_PHASE, NT, P, J_DIM)).astype(
        ml_dtypes.bfloat16)
    in_maps = []
    for c in range(N_CORES):
        xs = x_red[c * B_LOCAL:(c + 1) * B_LOCAL, :]        # [b, i]
        in_maps.append({"xT": np.ascontiguousarray(xs.T), "w": w_pack})
    return in_maps


def kernel(x, fouriercoeffs):
    nc = get_nc()
    in_maps = prepare_inputs(x, fouriercoeffs)
    res = run_bass_kernel_spmd(nc, in_maps, core_ids=list(range(N_CORES)))
    y = np.concatenate([r["yT"].T for r in res.results], axis=0)
    return np.ascontiguousarray(y, dtype=np.float32)


# revision 7
# speedup vs baseline: 1.0587x; 1.0587x over previous
"""NaiveFourierKANLayer on 8 Trainium2 NeuronCores (Bass/Tile).

y[b,j] = sum_{i,g} cos(g*x[b,i]) * W[0,j,i,g] + sin(g*x[b,i]) * W[1,j,i,g]

Strategy (data-parallel over batch, 1024 rows/core):
- Host: range-reduce x to [-pi,pi] (g integer => g*x mod 2pi preserved),
  transpose to x^T [i,b]; pack W bf16 as per-contraction-tile slabs
  [phase, n, ki, all-j].
- Device per core: theta_g chain via one fused custom DVE op per harmonic
  (tensor-add + period-wrap), sin+cos args evaluated by a single ScalarE Sin
  pass per harmonic (bf16 out); TensorE runs 2048 accumulating matmuls
  (K=128, M=128 j, N=512 b) n-outer/j-inner into 8 PSUM banks; y^T out f32.
"""
import numpy as np
import ml_dtypes

import concourse.mybir as mybir
import concourse.tile as tile
from concourse import bacc
from concourse.bass_utils import run_bass_kernel_spmd

# ---- runtime-registered custom DVE op: out = wrap(in0 + in1, [-b, b]) ------
# Mirrors concourse's ADD_RANGE_WRAP with a tensor (Src1) shift instead of the
# scalar C0 -- fuses the harmonic chain's tensor_add + add_range_wrap into one
# DVE pass. Registered into concourse.dve_ops at import (idempotent).
from concourse import dve_ops as _dve_ops
from concourse.dve_ops import DveOp as _DveOp
from concourse.dve_spec import C1 as _C1, C2 as _C2, Spec as _Spec, \
    Src0 as _Src0, Src1 as _Src1, lower as _dve_lower, _has_src1
from concourse.dve_uop import DveOpSpec as _DveOpSpec

_y = _Src0 + _Src1
ADD_T_RANGE_WRAP = _DveOp(
    "ADD_T_RANGE_WRAP",
    _Spec(
        body=_y + _C2 * ((_y < -_C1) - (_y > _C1)),
        reference=lambda in0, in1, s0, s1, imm2: (in0 + in1)
        + imm2 * (((in0 + in1) < -s1).astype(np.float32)
                  - ((in0 + in1) > s1).astype(np.float32)),
    ),
    subdim=False,
    uops_sha={},
)


def _register_fused_op():
    already = ADD_T_RANGE_WRAP.name in _dve_ops._SUB_OPCODE_FOR_NAME
    if not already:
        _dve_ops.OPS.append(ADD_T_RANGE_WRAP)
        _dve_ops.CUSTOM_DVE_SPECS[ADD_T_RANGE_WRAP.name] = ADD_T_RANGE_WRAP.spec
        row = _dve_ops._CUSTOM_DVE_ROW_BASE + len(_dve_ops.OPS) - 1
        assert row < 0x20, "custom-DVE row field overflow"
        _dve_ops._SUB_OPCODE_FOR_NAME[ADD_T_RANGE_WRAP.name] = row
    row = _dve_ops._SUB_OPCODE_FOR_NAME[ADD_T_RANGE_WRAP.name]
    for ver in ("v3", "v4"):
        spec = _DveOpSpec(
            name=ADD_T_RANGE_WRAP.name, opcode=row,
            uops=_dve_lower(ADD_T_RANGE_WRAP.spec, ver=ver),
            rd1_en=_has_src1(ADD_T_RANGE_WRAP.spec),
        )
        ADD_T_RANGE_WRAP.uops_sha[ver] = spec.sha(ver)


_register_fused_op()


def _add_t_range_wrap(nc, out, in0, in1, bound, period):
    return nc.vector._custom_dve(
        ADD_T_RANGE_WRAP, out=out, in0=in0, in1=in1, s1=bound, imm2=period)

N_CORES = 8
B_TOTAL = 8192
B_LOCAL = B_TOTAL // N_CORES   # 1024
I_DIM = 1024
J_DIM = 1024
G = 8
P = 128
NB_HALF = 2                    # batch halves per core (512 cols each)
BH = B_LOCAL // NB_HALF        # 512
N_PHASE = 2                    # contraction phases (i-tiles 0-3, 4-7)
II_PER_PHASE = I_DIM // P // N_PHASE   # 4
NT = II_PER_PHASE * G * 2      # 64 contraction tiles per phase
NJ = J_DIM // P                # 8

PI = float(np.pi)
TWO_PI = float(2 * np.pi)
AF = mybir.ActivationFunctionType
BF16 = mybir.dt.bfloat16
F32 = mybir.dt.float32

TH_BUFS = 6
WP_BUFS = 16
_NC_CACHE = {}


def _emit_mms(nc, ps_tiles, wp, w_d, const_w, ft, bh, ph, n, variant):
    if variant == "mm_nodma":
        wt = const_w
    else:
        wt = wp.tile([P, J_DIM], BF16, tag="w", bufs=WP_BUFS,
                     name=f"w_{bh}_{ph}_{n}")
        nc.sync.dma_start(out=wt, in_=w_d[ph, n])
    for jt in range(NJ):
        nc.tensor.matmul(
            ps_tiles[jt], wt[:, jt * P:(jt + 1) * P], ft,
            start=(ph == 0 and n == 0),
            stop=(ph == N_PHASE - 1 and n == NT - 1),
        )


def _body(nc, tc, xp, wp, fp, tp, op, pp, xT_d, w_d, yT_d, variant="full"):
    xt_tiles = []
    for it in range(I_DIM // P):
        xti = xp.tile([P, B_LOCAL], F32, tag=f"x{it}", name=f"x{it}")
        nc.sync.dma_start(out=xti, in_=xT_d[it * P:(it + 1) * P, :])
        xt_tiles.append(xti)
    const_f = None
    const_w = None
    if variant in ("mm_only", "mm_nodma"):
        const_f = xp.tile([P, BH], BF16, name="const_f")
        nc.sync.dma_start(out=const_f, in_=w_d[0, 0, :, 0:BH])
    if variant == "mm_nodma":
        const_w = xp.tile([P, J_DIM], BF16, name="const_w")
        nc.vector.memset(const_w, 0.5)

    for bh in range(NB_HALF):
        bs = slice(bh * BH, (bh + 1) * BH)
        ps_tiles = []
        for jt in range(NJ):
            ps = pp.tile([P, BH], F32, tag=f"ps{jt}", name=f"ps{jt}_{bh}")
            ps_tiles.append(ps)

        for ph in range(N_PHASE):
            # n-outer schedule: feature tile n is consumed by 8 back-to-back
            # matmuls (one per j-tile) right after production, so feat slots
            # recycle fast and ACT/DVE stay ahead of PE across boundaries.
            for ii in range(II_PER_PHASE):
                it = ph * II_PER_PHASE + ii
                xs = xt_tiles[it][:, bs]
                if variant in ("mm_only", "mm_nodma"):
                    for n in range(ii * 2 * G, (ii + 1) * 2 * G):
                        _emit_mms(nc, ps_tiles, wp, w_d, const_w, const_f,
                                  bh, ph, n, variant)
                    continue
                for g in range(1, G + 1):
                    nbase = ii * (G * 2) + (g - 1) * 2
                    if g == 1:
                        tcos = tp.tile([P, BH], F32, tag="tc", bufs=3,
                                       name=f"tc_{bh}_{it}")
                        nc.vector.add_range_wrap(tcos, xs, PI / 2, PI, TWO_PI)
                        fc = fp.tile([P, BH], BF16, tag=f"fc{ii}",
                                     name=f"fc_{bh}_{it}")
                        nc.scalar.activation(out=fc, in_=tcos, func=AF.Sin)
                        fs = fp.tile([P, BH], BF16, tag=f"fs{ii}",
                                     name=f"fs_{bh}_{it}")
                        nc.scalar.activation(out=fs, in_=xs, func=AF.Sin)
                        th_prev = xs
                        f_cos, f_sin = fc, fs
                    else:
                        tharg = tp.tile([P, 2, BH], F32, tag="th", bufs=TH_BUFS,
                                        name=f"th_{bh}_{it}_{g}")
                        _add_t_range_wrap(
                            nc, tharg[:, 1, :], th_prev, xs, PI, TWO_PI)
                        nc.vector.add_range_wrap(
                            tharg[:, 0, :], tharg[:, 1, :], PI / 2, PI, TWO_PI)
                        f = fp.tile([P, 2, BH], BF16, tag=f"f{ii}_{g}",
                                    name=f"f_{bh}_{it}_{g}")
                        nc.scalar.activation(out=f, in_=tharg, func=AF.Sin)
                        th_prev = tharg[:, 1, :]
                        f_cos, f_sin = f[:, 0, :], f[:, 1, :]
                    if variant == "feats_only":
                        continue
                    _emit_mms(nc, ps_tiles, wp, w_d, const_w, f_cos,
                              bh, ph, nbase, variant)
                    _emit_mms(nc, ps_tiles, wp, w_d, const_w, f_sin,
                              bh, ph, nbase + 1, variant)

        if variant != "feats_only":
            for jt in range(NJ):
                ot = op.tile([P, BH], F32, tag="out", name=f"ot_{bh}_{jt}")
                nc.scalar.copy(out=ot, in_=ps_tiles[jt])
                nc.sync.dma_start(out=yT_d[jt * P:(jt + 1) * P, bs], in_=ot)


def _build_nc(loop_reps=None, variant="full"):
    nc = bacc.Bacc("TRN2", debug=False, num_devices=N_CORES)
    xT_d = nc.dram_tensor("xT", [I_DIM, B_LOCAL], F32, kind="ExternalInput").ap()
    w_d = nc.dram_tensor("w", [N_PHASE, NT, P, J_DIM], BF16, kind="ExternalInput").ap()
    yT_d = nc.dram_tensor("yT", [J_DIM, B_LOCAL], F32, kind="ExternalOutput").ap()

    with tile.TileContext(nc) as tc:
        with tc.tile_pool(name="xp", bufs=1) as xp, \
             tc.tile_pool(name="wp", bufs=3) as wp, \
             tc.tile_pool(name="fp", bufs=1) as fp, \
             tc.tile_pool(name="tp", bufs=1) as tp, \
             tc.tile_pool(name="op", bufs=4) as op, \
             tc.tile_pool(name="pp", bufs=1, space="PSUM") as pp:
            pools = (xp, wp, fp, tp, op, pp)
            if loop_reps is None:
                _body(nc, tc, *pools, xT_d, w_d, yT_d, variant=variant)
            else:
                with tc.For_i(0, loop_reps, 1):
                    _body(nc, tc, *pools, xT_d, w_d, yT_d, variant=variant)

    nc.compile()
    return nc


def get_nc(loop_reps=None, variant="full"):
    key = (loop_reps, variant)
    if key not in _NC_CACHE:
        _NC_CACHE[key] = _build_nc(loop_reps, variant)
    return _NC_CACHE[key]


def prepare_inputs(x, fouriercoeffs):
    """Host-side prep: range-reduce + transpose x, pack W to bf16 slabs."""
    x = np.asarray(x, dtype=np.float32)
    w = np.asarray(fouriercoeffs, dtype=np.float32)
    x64 = x.astype(np.float64)
    x_red = (x64 - TWO_PI * np.round(x64 / TWO_PI)).astype(np.float32)
    # pack: coeffs [t, j, i, g] -> [ph, n=(ii,g,t), ki, j]
    a = w.reshape(2, J_DIM, N_PHASE, II_PER_PHASE, P, G)  # [t, j, ph, ii, ki, g]
    a = a.transpose(2, 3, 5, 0, 4, 1)                     # [ph, ii, g, t, ki, j]
    w_pack = np.ascontiguousarray(a.reshape(N_PHASE, NT, P, J_DIM)).astype(
        ml_dtypes.bfloat16)
    in_maps = []
    for c in range(N_CORES):
        xs = x_red[c * B_LOCAL:(c + 1) * B_LOCAL, :]        # [b, i]
        in_maps.append({"xT": np.ascontiguousarray(xs.T), "w": w_pack})
    return in_maps


def kernel(x, fouriercoeffs):
    nc = get_nc()
    in_maps = prepare_inputs(x, fouriercoeffs)
    res = run_bass_kernel_spmd(nc, in_maps, core_ids=list(range(N_CORES)))
    y = np.concatenate([r["yT"].T for r in res.results], axis=0)
    return np.ascontiguousarray(y, dtype=np.float32)
